# revision 1
# baseline (speedup 1.0000x reference)
"""Trainium2 Bass kernel for LocalSemanticAlignment (sparse_attention).

Pipeline (reference semantics):
  masks   = parse[:,1:] downsampled 256->64 (nearest, stride-4)
  ufb     = bilinear-AC downsample of unalign_fb to 64x64        (host)
  fan/fbn = per-channel-centered, per-column L2-normalized fa/fb (host)
  S[q,p]  = fbn^T fan                                            (device, bf16 matmul)
  per class k: w_k = where(mask_b[q], exp(alpha*S - C), exp(-C)) masked
  softmax over q (shift C is exact; see CSHIFT)
  warped_k = ufb @ softmax  ->  combined over k with mask_a / counts
  output  = bilinear-AC upsample of aligned to 256x256           (host)

Key identity used on device: w_k = mask_b[q]*exp(aS-C) + (1-mask_b[q])e^-C, so
  numer_k = (ufb*mask_b[k])^T @ E + const_k,  denom_k = mask_b[k]^T @ E + z_k
with E = exp(aS - C) shared across classes. All masking is folded into the
stationary (lhsT) operands, so the device loop is pure matmul + Exp. The
numerators/denominators (195 x PC per core) are shipped back and the final
divide+combine (trivial) happens on the host.

Sharding: output columns (p axis) are split across the 8 cores; every core
holds full fbn/ufb (keys/values) and computes its shard end-to-end. No
collectives.

Perf structure (evolved from a 56us baseline to ~43us):
 - Numerator matmuls run TRANSPOSED (E tile stationary, ucomb moving,
   output [p,195] per 128-p block): 4 N=195 matmuls per q-tile instead of
   2 N=464 ones — ~2us less PE streaming, and the output shrinks to
   [128,780]. Two p-blocks share each PSUM bank; since start=True clears a
   whole bank, the accumulators are DVE-zeroed up front and every numer
   matmul accumulates with start=False.
 - Dead-column removal: p-columns whose fa_parse has no active class produce
   zero output (~12% of 4096 for these inputs); the host permutes them out
   and each core computes PC=464 live columns instead of 512. Shrinks every
   matmul's moving dim, the fan DMA, and the output.
 - fbn/fan in bf16: halves the dominant DMA; S-matmul LDWEIGHTS gets FWL.
   Host-simulated rel err with bf16 features is ~1.2e-2 (gate 2e-2).
 - Software-pipelined issue order: S matmuls for double-tile dt+2 are issued
   before the numer matmuls of dt, so the PE always has >1.1us of
   independent work while an exp is in flight (measured zero-wait ACT chain).
 - Per-half exps ([128,464] each) keep each PSUM-bank write within a bank
   and halve the exp latency ahead of the first/last numer matmuls.
 - Queue plan: first-dependency set (fbn chunk0 + fan) heads the SP HWDGE
   queue; ucomb chunk0 + chunk1 + the Exp-table-preloading dummy activation
   ride the ACT queue; everything else streams on SP. One [128,928] output
   DMA (a [67,x] DMA lands on few engines at ~17GB/s - measured twice).
 - Warmup matmuls sized to the first-chunk ETA keep the PE HAM clock-gate
   at 8/8 when real work starts.
"""

import numpy as np
import ml_dtypes

import concourse.bass as bass
import concourse.bacc as bacc
import concourse.mybir as mybir
from concourse import tile
from concourse.bass_utils import run_bass_kernel_spmd

ALPHA = 100.0
# global logit shift: exp(alpha*S - CSHIFT) everywhere, with the "+1" weights
# of masked-out keys scaled by exp(-CSHIFT) on the host (vks/zk). Softmax is
# shift-invariant so this is exact; it keeps exp() in f32/bf16 range for
# logits up to CSHIFT+88 (observed max ~90).
CSHIFT = 30.0   # 30 (not 60): with E as the STATIONARY matmul operand, a
                # column whose max logit sits ~26 below the shift had its
                # tiny bf16 weights (~e-12) vanish in the PE weight path,
                # collapsing that column's softmax. At 30 every column's
                # dominant weights are >= ~1 while the max logit (~90)
                # keeps exp(90-30)=e60 well inside bf16/f32 range.
N_CORES = 8
HW = 4096          # 64*64 spatial positions at feature resolution
PC = 456           # live p columns per core (8*456=3648 >= the 3600 live)
NQT = HW // 128    # 32 q-tiles of 128
UC = 195           # ucomb cols per q-tile: U0|U1|U2|mb0|mb1|mb2 = 64*3+3

F32 = mybir.dt.float32
BF16 = mybir.dt.bfloat16


def _interp_bilinear_ac(x, size):
    """torch F.interpolate bilinear align_corners=True; x: (C,H,W) float32."""
    x = np.ascontiguousarray(x, np.float32)
    H, W = x.shape[-2], x.shape[-1]
    h, w = size

    def coords(n_out, n_in):
        if n_out == 1:
            return np.zeros((1,), np.float32)
        return np.arange(n_out, dtype=np.float32) * np.float32((n_in - 1) / (n_out - 1))

    ry, rx = coords(h, H), coords(w, W)
    y0 = np.floor(ry).astype(np.int32)
    x0 = np.floor(rx).astype(np.int32)
    y1 = np.clip(y0 + 1, 0, H - 1)
    x1 = np.clip(x0 + 1, 0, W - 1)
    wy = (ry - y0.astype(np.float32))[None, :, None]
    wx = (rx - x0.astype(np.float32))[None, None, :]
    rows = x[:, y0, :] * (1.0 - wy) + x[:, y1, :] * wy
    return (rows[:, :, x0] * (1.0 - wx) + rows[:, :, x1] * wx).astype(np.float32)


_NC_CACHE = {}

# q-tiles per DMA chunk; chunk0 split across both queues with fan.
# Starting smaller/earlier was measured NET-WORSE: the cold DMA engines
# deliver ~1 q-tile per 0.8us early on, so an earlier compute start just
# converts startup wait into mid-stream supply stalls.
CHUNKS = [4, 4, 6, 6, 6, 6]
NWARM = 44   # PE warm-up matmuls (N=128, ~115ns cold each). Must give >3.4us
             # of CONTINUOUS PE busy (the HAM clock-gate needs one full busy
             # window to release 4/8->8/8; 22 warmups measured NOT warming,
             # leaving the first 6us of real matmuls at 1.2GHz) and bridge
             # the ~4.7us until fbn chunk0 + fan land.


def _build_program():
    if "nc" in _NC_CACHE:
        return _NC_CACHE["nc"]

    nc = bacc.Bacc("TRN2", target_bir_lowering=False, debug=False,
                   num_devices=N_CORES)

    fbn_d = nc.dram_tensor("fbn", [2, 128, HW], BF16, kind="ExternalInput").ap()
    fan_d = nc.dram_tensor("fan", [2, 128, PC], BF16, kind="ExternalInput").ap()
    uc_d = nc.dram_tensor("ucomb", [128, NQT * UC], BF16, kind="ExternalInput").ap()
    out_d = nc.dram_tensor("out_nd", [128, 4 * UC], BF16,
                           kind="ExternalOutput").ap()

    EXP = mybir.ActivationFunctionType.Exp

    with tile.TileContext(nc) as tc:
        with (
            tc.tile_pool(name="io", bufs=1) as io,
            tc.tile_pool(name="big", bufs=1) as big,
            tc.tile_pool(name="expp", bufs=3) as expp,
            tc.tile_pool(name="spsum", bufs=3, space="PSUM") as spsum,
            tc.tile_pool(name="npsum", bufs=1, space="PSUM") as npsum,
            tc.tile_pool(name="fin", bufs=1) as fin,
        ):
            # numerators accumulate TRANSPOSED: out[p, 195] per 128-p block
            # (E tile stationary, ucomb moving) — 4 N=195 matmuls per q-tile
            # instead of 2 N=464 ones (~2us less PE streaming). Two p-blocks
            # pack into each 2KB PSUM bank (cols 0:195 and 256:451) so the
            # S pipeline keeps its 3 double-buffers: 3*2 + 2 = 8 banks.
            n1_ps = npsum.tile([128, 512], F32, tag="n1")   # p-blocks 0,1
            n2_ps = npsum.tile([128, 512], F32, tag="n2")   # p-blocks 2,3

            wz_sb = io.tile([128, 128], BF16, tag="wz")
            nc.vector.memset(wz_sb[:], 0.0)
            cb_sb = io.tile([128, 1], F32, tag="cb")
            nc.vector.memset(cb_sb[:], -CSHIFT)

            # PE warm-up: trips the HAM activity window so real matmuls run
            # at 2.4 GHz. Scribbles into n1_ps, which the first real
            # accumulation resets (start=True).
            for _ in range(NWARM):
                nc.tensor.matmul(n1_ps[:, 0:128], wz_sb[:], wz_sb[:],
                                 start=True, stop=True)

            # zero the numer accumulator banks (warmups scribbled n1 and
            # n2 starts uninitialized; the real matmuls never clear)
            nc.vector.memset(n1_ps[:], 0.0)
            nc.vector.memset(n2_ps[:], 0.0)

            fan_sb = [io.tile([128, PC], BF16, tag=f"fan{c}",
                              name=f"fan_sb{c}") for c in range(2)]
            fbn_sb = [big.tile([128, HW], BF16, tag=f"fbn{c}", name=f"fbn_sb{c}")
                      for c in range(2)]
            uc_sb = big.tile([128, NQT * UC], BF16, tag="ucomb")

            def fbn_dma(eng, ci):
                q0 = sum(CHUNKS[:ci])
                qs = slice(q0 * 128, (q0 + CHUNKS[ci]) * 128)
                eng.dma_start(fbn_sb[0][:, qs], fbn_d[0][:, qs])
                eng.dma_start(fbn_sb[1][:, qs], fbn_d[1][:, qs])

            def uc_dma(eng, ci):
                q0 = sum(CHUNKS[:ci])
                us = slice(q0 * UC, (q0 + CHUNKS[ci]) * UC)
                eng.dma_start(uc_sb[:, us], uc_d[:, us])

            # The first S-matmul's dependency set split across BOTH HWDGE
            # queues so its ~490KB transfers in parallel (serializing it on
            # one queue was measured costing ~2.5us): SP gets fbn[0]c0+fan0,
            # ACT gets fbn[1]c0+fan1+ucomb c0 then the Exp-table preload.
            # Everything else streams on SP (~0.6us issue cost each; SP is
            # otherwise idle and the ACT queue must stay clear for the exps).
            q0s = slice(0, CHUNKS[0] * 128)
            u0s = slice(0, CHUNKS[0] * UC)
            nc.sync.dma_start(fbn_sb[0][:, q0s], fbn_d[0][:, q0s])
            nc.scalar.dma_start(fbn_sb[1][:, q0s], fbn_d[1][:, q0s])
            nc.sync.dma_start(fan_sb[0][:], fan_d[0])
            nc.scalar.dma_start(fan_sb[1][:], fan_d[1])
            nc.scalar.dma_start(uc_sb[:, u0s], uc_d[:, u0s])
            dum_sb = io.tile([128, 1], BF16, tag="dum")
            nc.scalar.activation(dum_sb[:], cb_sb[:], EXP, scale=1.0)
            for ci in range(1, len(CHUNKS)):
                fbn_dma(nc.sync, ci)
                uc_dma(nc.sync, ci)

            ND = NQT // 2
            PIPE = 2   # S-matmul lookahead (double-tiles) past the exp stage

            def s_mms(dt, s2_ps):
                for h in range(2):
                    t = 2 * dt + h
                    qs = slice(t * 128, (t + 1) * 128)
                    hs = slice(h * 512, h * 512 + PC)
                    nc.tensor.matmul(s2_ps[:, hs], fbn_sb[0][:, qs],
                                     fan_sb[0][:], start=True, stop=False)
                    nc.tensor.matmul(s2_ps[:, hs], fbn_sb[1][:, qs],
                                     fan_sb[1][:], start=False, stop=True)

            # software pipeline: keep PIPE double-tiles of S matmuls queued
            # ahead of the exp stage, so the PE always has independent work
            # while an exp is in flight (without this the PE idles
            # ~0.3-2.5us at every other exp).
            s_ps = {}
            for dt in range(PIPE):
                s_ps[dt] = spsum.tile([128, 1024], F32, tag="s",
                                      name=f"s2_{dt}")
                s_mms(dt, s_ps[dt])
            for dt in range(ND):
                if dt + PIPE < ND:
                    s_ps[dt + PIPE] = spsum.tile([128, 1024], F32, tag="s",
                                                 name=f"s2_{dt + PIPE}")
                    s_mms(dt + PIPE, s_ps[dt + PIPE])
                e2_sb = expp.tile([128, 2 * PC], BF16, tag="e")
                sp = s_ps.pop(dt)
                for h in range(2):
                    t = 2 * dt + h
                    es = slice(h * PC, (h + 1) * PC)
                    nc.scalar.activation(e2_sb[:, es],
                                         sp[:, h * 512:h * 512 + PC], EXP,
                                         scale=ALPHA, bias=cb_sb[:])
                    uct = uc_sb[:, t * UC:t * UC + UC]
                    last = (t == NQT - 1)
                    # start=True clears the WHOLE psum bank, which would
                    # wipe the sibling accumulation group sharing it — so
                    # the numer banks are DVE-memset to zero up front and
                    # every matmul accumulates (start=False).
                    for b in range(4):
                        bw = min(128, PC - b * 128)          # 128,128,128,80
                        eb = e2_sb[:, h * PC + b * 128:h * PC + b * 128 + bw]
                        ps = (n1_ps if b < 2 else n2_ps)
                        cs = slice(0, UC) if b % 2 == 0 else slice(256, 256 + UC)
                        nc.tensor.matmul(ps[0:bw, cs], eb, uct,
                                         start=False, stop=last)

            # ship raw numerators + denominators to the host in bf16 (the
            # divide on the host adds ~0.2% error, still well under gate);
            # two copies on DVE, two on ACT (idle by then) so they run
            # concurrently; one [128,780] DMA keeps the transfer spread
            # over all 16 DMA engines.
            o_sb = fin.tile([128, 4 * UC], BF16, tag="o")
            nc.vector.tensor_copy(o_sb[:, 0:UC], n1_ps[:, 0:UC])
            nc.scalar.copy(o_sb[:, UC:2 * UC], n1_ps[:, 256:256 + UC])
            # first half ships as soon as its two copies land; second half
            # rides the ACT HWDGE queue so the issues don't serialize
            nc.sync.dma_start(out_d[:, 0:2 * UC], o_sb[:, 0:2 * UC])
            nc.vector.tensor_copy(o_sb[:, 2 * UC:3 * UC], n2_ps[:, 0:UC])
            nc.vector.tensor_copy(o_sb[0:72, 3 * UC:4 * UC],
                                  n2_ps[0:72, 256:256 + UC])
            nc.scalar.dma_start(out_d[:, 2 * UC:4 * UC],
                                o_sb[:, 2 * UC:4 * UC])

    nc.compile()
    _NC_CACHE["nc"] = nc
    return nc


def _prep_inputs(unalign_fb, fa, fa_parse, fb, fb_parse):
    c2 = unalign_fb.shape[1]
    c = fa.shape[1]
    mask_a = (fa_parse[0, 1:, ::4, ::4].reshape(3, HW) != 0).astype(np.float32)
    mask_b = (fb_parse[0, 1:, ::4, ::4].reshape(3, HW) != 0).astype(np.float32)
    ufb = _interp_bilinear_ac(unalign_fb[0], (64, 64)).reshape(c2, HW)

    faf = np.ascontiguousarray(fa[0].reshape(c, HW), np.float32)
    fbf = np.ascontiguousarray(fb[0].reshape(c, HW), np.float32)
    faf = faf - faf.mean(axis=1, keepdims=True, dtype=np.float32)
    fbf = fbf - fbf.mean(axis=1, keepdims=True, dtype=np.float32)
    fan = faf / np.linalg.norm(faf, axis=0, keepdims=True)
    fbn = fbf / np.linalg.norm(fbf, axis=0, keepdims=True)

    # dead-column removal: p with no active class produce zero output; pack
    # the live ones (padded with repeats, ignored at scatter time)
    live = np.flatnonzero(mask_a.sum(axis=0) > 0)
    npad = N_CORES * PC
    assert live.size <= npad, f"live columns {live.size} > capacity {npad}"
    perm = np.concatenate([live, np.full(npad - live.size, live[0], np.int64)])

    # stationary operands for the numerator/denominator matmuls, tiled per
    # 128-q block: [U0|U1|U2|mb0|mb1|mb2] transposed to [q,cols]
    U = ufb[None] * mask_b[:, None, :]                     # (3,64,HW)
    ucomb = np.empty((128, NQT * UC), np.float32)
    Ut = U.transpose(2, 0, 1).reshape(HW, 3 * 64)          # (HW, 192) q-major
    mbt = mask_b.T                                         # (HW, 3)
    for t in range(NQT):
        qs = slice(t * 128, (t + 1) * 128)
        ucomb[:, t * UC:t * UC + 192] = Ut[qs]
        ucomb[:, t * UC + 192:t * UC + 195] = mbt[qs]
    ucomb = ucomb.astype(ml_dtypes.bfloat16)

    fbn3 = np.ascontiguousarray(fbn.reshape(2, 128, HW)).astype(ml_dtypes.bfloat16)
    fan_p = fan[:, perm].reshape(2, 128, npad).astype(ml_dtypes.bfloat16)
    in_maps = []
    for i in range(N_CORES):
        ps = slice(i * PC, (i + 1) * PC)
        in_maps.append({
            "fbn": fbn3,
            "fan": np.ascontiguousarray(fan_p[:, :, ps]),
            "ucomb": ucomb,
        })

    # host-epilogue constants
    esc = np.float32(np.exp(-CSHIFT))
    norm = np.maximum(mask_a.sum(axis=0), 1.0)
    ga = (mask_a / norm[None, :]).astype(np.float32)            # (3,HW)
    vks = (ufb @ (1.0 - mask_b).T).astype(np.float32) * esc     # (64,3)
    zk = ((1.0 - mask_b).sum(axis=1).astype(np.float32) * esc)  # (3,)
    return in_maps, (ga, vks, zk, live, perm)


def _run(inputs, trace=False, trace_cores=None):
    unalign_fb = np.asarray(inputs["unalign_fb"], np.float32)
    fa = np.asarray(inputs["fa"], np.float32)
    fa_parse = np.asarray(inputs["fa_parse"])
    fb = np.asarray(inputs["fb"], np.float32)
    fb_parse = np.asarray(inputs["fb_parse"])

    nc = _build_program()
    in_maps, (ga, vks, zk, live, perm) = _prep_inputs(
        unalign_fb, fa, fa_parse, fb, fb_parse)
    res = run_bass_kernel_spmd(nc, in_maps, core_ids=list(range(N_CORES)),
                               trace=trace, trace_cores=trace_cores)

    c2 = unalign_fb.shape[1]
    # per core: 4 p-blocks of [rows=p, 195] stacked -> (PC, 195)
    nd_all = np.concatenate(
        [np.concatenate([res.results[i]["out_nd"][0:min(128, PC - b * 128),
                                                  b * UC:(b + 1) * UC]
                         for b in range(4)])
         for i in range(N_CORES)]).astype(np.float32)      # (8*PC, 195)
    ga_p = ga[:, perm]
    combined = np.zeros((c2, N_CORES * PC), np.float32)
    for k in range(3):
        numer = nd_all[:, 64 * k:64 * k + 64].T + vks[:, k:k + 1]
        denom = nd_all[:, 192 + k] + zk[k]
        combined += (ga_p[k] / denom)[None, :] * numer
    aligned = np.zeros((c2, HW), np.float32)
    aligned[:, live] = combined[:, :live.size]
    out = _interp_bilinear_ac(aligned.reshape(c2, 64, 64), (256, 256))
    return out[None], res


def kernel(**inputs):
    out, _ = _run(inputs)
    return out



# revision 5
# speedup vs baseline: 1.1057x; 1.1057x over previous
"""Trainium2 Bass kernel for LocalSemanticAlignment (sparse_attention).

Pipeline (reference semantics):
  masks   = parse[:,1:] downsampled 256->64 (nearest, stride-4)
  ufb     = bilinear-AC downsample of unalign_fb to 64x64        (host)
  fan/fbn = per-channel-centered, per-column L2-normalized fa/fb (host)
  S[q,p]  = fbn^T fan                                            (device, bf16 matmul)
  per class k: w_k = where(mask_b[q], exp(alpha*S - C), exp(-C))
  softmax over q; warped_k = ufb @ softmax; combine over k with mask_a
  output  = bilinear-AC upsample of aligned to 256x256           (host)

Key identities used on device:
 - w_k = mask_b[k,q]*exp(aS-C) + (1-mask_b[k,q])e^-C, so the e^-C part is a
   per-(k,p) constant handled on the host (vks/zk); the device only needs
   E = exp(aS - C) summed against ufb over the q's where mask_b[k,q]=1.
 - GROUP DECOMPOSITION: each q belongs to exactly one of 7 nonzero mask_b
   bit-patterns ("groups"). numer_k = sum over groups g containing k of
   numer_g, with numer_g = sum_{q in g} ufb[:,q] E[q,p] (plus the ones-col
   giving denom_g). So the numerator moving operand is 65 cols (64 ufb + 1
   ones) instead of 3*64+3=195 — 3x less PE streaming — and q's in NO group
   (~1/8) are dropped entirely (28 q-tiles instead of 32 for S + exp + DMA).

Device loop per q-tile-pair: 4 S matmuls (bf16) -> ONE exp over both tiles'
PSUM banks (halves the ACT fixed overhead; ~204ns/instr) -> per tile, per
128-p block, per group: transposed numer matmul (E stationary, M=[ufb|1]
moving, 65 cols) accumulated into per-group buckets (7*65=455 f32 cols per
p-block bank; 4 banks). Software pipeline: S(i+1) issues while exp(i) runs;
numers(i) follow and never wait (exp 1010ns < S-pair 1520ns).

PSUM: 2 S pair-buffers (2 banks each) + 4 numer banks = 8 exactly.

DMA: 3 input HWDGE queues (Sync: fbn half0, Vector: fbn half1, GpSimd:
fan + M) — each dma_start costs ~0.65us of issue time on its engine, and a
queue sustains only ~110-130GB/s, so v1's 2-queue plan supply-stalled the
PE. Scalar's queue stays clear for the exps; output rides Sync+Scalar at
the end. Warmup matmuls (HAM clock gate releases after ~3us of continuous
PE busy) target s-psum buf0, which the first real S matmul's start=True
reset anyway, so the numer-bank memsets don't wait on them.

Sharding: output columns (p) split across 8 cores; each core holds full
fbn/M (keys/values) and computes its shard end-to-end. No collectives.
Numerator/denominator buckets ship raw (bf16) and the final
divide+combine (trivial) happens on the host.
"""

import numpy as np
import ml_dtypes

import concourse.bass as bass
import concourse.bacc as bacc
import concourse.mybir as mybir
from concourse import tile
from concourse.bass_utils import run_bass_kernel_spmd

ALPHA = 100.0
# global logit shift: exp(alpha*S - CSHIFT); softmax-shift-exact, the "+1"
# weights of masked-out q's are scaled by exp(-CSHIFT) on the host (vks/zk).
# 30 (not 60): E is the STATIONARY numer-matmul operand in bf16; at 60 a
# column whose max logit sits far below the shift had its tiny weights
# vanish in the PE weight path. At 30 the max logit (~90) keeps
# exp(90-30)=e60 inside bf16/f32 range.
CSHIFT = 30.0
N_CORES = 8
HW = 4096
NWARM = 36   # PE warm-up matmuls (~107ns each at the pre-ramp clock):
             # bridge the first-chunk DMA wait AND give the HAM clock gate
             # its continuous-busy window so real matmuls run at 2.4GHz.

F32 = mybir.dt.float32
BF16 = mybir.dt.bfloat16
EXP = mybir.ActivationFunctionType.Exp


def _interp_bilinear_ac(x, size):
    """torch F.interpolate bilinear align_corners=True; x: (C,H,W) float32."""
    x = np.ascontiguousarray(x, np.float32)
    H, W = x.shape[-2], x.shape[-1]
    h, w = size

    def coords(n_out, n_in):
        if n_out == 1:
            return np.zeros((1,), np.float32)
        return np.arange(n_out, dtype=np.float32) * np.float32((n_in - 1) / (n_out - 1))

    ry, rx = coords(h, H), coords(w, W)
    y0 = np.floor(ry).astype(np.int32)
    x0 = np.floor(rx).astype(np.int32)
    y1 = np.clip(y0 + 1, 0, H - 1)
    x1 = np.clip(x0 + 1, 0, W - 1)
    wy = (ry - y0.astype(np.float32))[None, :, None]
    wx = (rx - x0.astype(np.float32))[None, None, :]
    rows = x[:, y0, :] * (1.0 - wy) + x[:, y1, :] * wy
    return (rows[:, :, x0] * (1.0 - wx) + rows[:, :, x1] * wx).astype(np.float32)


_NC_CACHE = {}


def _build_program(key, NQ, PC, sched, fbn_chunks, m_chunks, MCOLS):
    """sched: tuple per tile of (groups tuple); groups are 1..7.
    fbn_chunks: tile counts per fbn DMA chunk. m_chunks: (tile0, ntiles)
    chunking of the M buffer (column offsets derived from sched)."""
    if key in _NC_CACHE:
        return _NC_CACHE[key]

    nc = bacc.Bacc("TRN2", target_bir_lowering=False, debug=False,
                   num_devices=N_CORES)

    fbn_d = nc.dram_tensor("fbn", [2, 128, NQ * 128], BF16,
                           kind="ExternalInput").ap()
    fan_d = nc.dram_tensor("fan", [2, 128, PC], BF16, kind="ExternalInput").ap()
    m_d = nc.dram_tensor("mcomb", [128, MCOLS], BF16, kind="ExternalInput").ap()
    out_d = nc.dram_tensor("out_nd", [128, 4 * 455], BF16,
                           kind="ExternalOutput").ap()

    NB = (PC + 127) // 128          # p blocks (4 for PC<=512)
    NPAIR = (NQ + 1) // 2

    # M column offset per (tile, group)
    moff = {}
    c = 0
    for t, groups in enumerate(sched):
        for g in groups:
            moff[(t, g)] = c
            c += 65
    assert c == MCOLS

    # per-tile M column ranges (for chunked DMA)
    tile_m0 = []
    c = 0
    for t, groups in enumerate(sched):
        tile_m0.append(c)
        c += 65 * len(groups)
    tile_m0.append(c)

    with tile.TileContext(nc) as tc:
        with (
            tc.tile_pool(name="io", bufs=1) as io,
            tc.tile_pool(name="big", bufs=1) as big,
            tc.tile_pool(name="expp", bufs=3) as expp,
            tc.tile_pool(name="spsum", bufs=2, space="PSUM") as spsum,
            tc.tile_pool(name="npsum", bufs=1, space="PSUM") as npsum,
            tc.tile_pool(name="fin", bufs=1) as fin,
        ):
            # numer buckets: one bank per 128-p block; cols g*65..g*65+65
            # hold group g's [64 ufb numer | 1 denom] for that block's p rows
            nb = [npsum.tile([128, 512], F32, tag=f"nb{b}", name=f"nb{b}")
                  for b in range(NB)]

            wz_sb = io.tile([128, 128], BF16, tag="wz")
            nc.vector.memset(wz_sb[:], 0.0)
            cb_sb = io.tile([128, 1], F32, tag="cb")
            nc.vector.memset(cb_sb[:], -CSHIFT)

            # S psum pair-buffers allocated BEFORE warmups so the warmup
            # scribbles land in buf0 (cleared by s_mms(0)'s start=True).
            s_ps = {}
            s_ps[0] = spsum.tile([128, 1024], F32, tag="s", name="s_0")

            for _ in range(NWARM):
                nc.tensor.matmul(s_ps[0][:, 0:128], wz_sb[:], wz_sb[:],
                                 start=True, stop=True)

            # zero the numer accumulator banks (matmuls accumulate with
            # start=False throughout; start=True would clear a whole bank
            # and wipe sibling groups). GPSIMD cannot touch PSUM -> DVE.
            for b in range(NB):
                nc.vector.memset(nb[b][:], 0.0)

            fan_sb = [io.tile([128, PC], BF16, tag=f"fan{c2}",
                              name=f"fan_sb{c2}") for c2 in range(2)]
            fbn_sb = [big.tile([128, NQ * 128], BF16, tag=f"fbn{c2}",
                               name=f"fbn_sb{c2}") for c2 in range(2)]
            m_sb = big.tile([128, MCOLS], BF16, tag="mcomb")

            # --- DMA issue plan: the two HWDGE queues (SP + ACT) ---
            # Sync: fan[0] then interleaved fbn half chunks (idle engine,
            # ~0.65us issue each). Scalar: fan[1], M c0, the exp-table
            # preload, remaining M chunks — all before its exp stream.
            def fbn_dma(eng, half, ci):
                q0 = sum(fbn_chunks[:ci])
                qs = slice(q0 * 128, (q0 + fbn_chunks[ci]) * 128)
                eng.dma_start(fbn_sb[half][:, qs], fbn_d[half][:, qs])

            def m_dma(eng, mi):
                t0, nt = m_chunks[mi]
                ms = slice(tile_m0[t0], tile_m0[min(t0 + nt, NQ)])
                eng.dma_start(m_sb[:, ms], m_d[:, ms])

            nc.sync.dma_start(fan_sb[0][:], fan_d[0])
            nc.scalar.dma_start(fan_sb[1][:], fan_d[1])
            for ci in range(len(fbn_chunks)):
                fbn_dma(nc.sync, 0, ci)
                fbn_dma(nc.sync, 1, ci)
            m_dma(nc.scalar, 0)
            dum_sb = io.tile([128, 1], BF16, tag="dum")
            nc.scalar.activation(dum_sb[:], cb_sb[:], EXP, scale=1.0)
            for mi in range(1, len(m_chunks)):
                m_dma(nc.scalar, mi)

            def s_mms(i, sp):
                for h in range(2):
                    t = 2 * i + h
                    if t >= NQ:
                        return
                    qs = slice(t * 128, (t + 1) * 128)
                    hs = slice(h * 512, h * 512 + PC)
                    nc.tensor.matmul(sp[:, hs], fbn_sb[0][:, qs],
                                     fan_sb[0][:], start=True, stop=False)
                    nc.tensor.matmul(sp[:, hs], fbn_sb[1][:, qs],
                                     fan_sb[1][:], start=False, stop=True)

            def numers(i, e_sb):
                for h in range(2):
                    t = 2 * i + h
                    if t >= NQ:
                        return
                    for b in range(NB):
                        bw = min(128, PC - b * 128)
                        eb = e_sb[:, h * 512 + b * 128:h * 512 + b * 128 + bw]
                        for g in sched[t]:
                            last = (t == NQ - 1 and g == sched[t][-1])
                            mc = moff[(t, g)]
                            nc.tensor.matmul(
                                nb[b][0:bw, (g - 1) * 65:(g - 1) * 65 + 65],
                                eb, m_sb[:, mc:mc + 65],
                                start=False, stop=last)

            # software pipeline: S(i+1) issues while exp(i) is in flight;
            # numers(i) read e(i) afterwards and never stall the PE.
            s_mms(0, s_ps[0])
            for i in range(NPAIR):
                if i + 1 < NPAIR:
                    s_ps[i + 1] = spsum.tile([128, 1024], F32, tag="s",
                                             name=f"s_{i + 1}")
                    s_mms(i + 1, s_ps[i + 1])
                sp = s_ps.pop(i)
                # one exp across the pair's two banks (cols between PC and
                # 512 were zeroed by the S start=True and exp to e^-30,
                # never read)
                W = 512 + PC if 2 * i + 1 < NQ else PC
                e_sb = expp.tile([128, 1024], BF16, tag="e")
                nc.scalar.activation(e_sb[:, 0:W], sp[:, 0:W], EXP,
                                     scale=ALPHA, bias=cb_sb[:])
                numers(i, e_sb)

            # ship raw numer/denom buckets in bf16; copies split across
            # DVE and Pool, out DMAs on Sync + Scalar (both idle by then)
            o_sb = fin.tile([128, 4 * 455], BF16, tag="o")
            nc.vector.tensor_copy(o_sb[:, 0:455], nb[0][:, 0:455])
            nc.scalar.copy(o_sb[:, 455:910], nb[1][:, 0:455])
            nc.sync.dma_start(out_d[:, 0:910], o_sb[:, 0:910])
            nc.vector.tensor_copy(o_sb[:, 910:1365], nb[2][:, 0:455])
            nc.scalar.copy(o_sb[:, 1365:1820], nb[3][:, 0:455])
            nc.scalar.dma_start(out_d[:, 910:1820], o_sb[:, 910:1820])

    nc.compile()
    _NC_CACHE[key] = nc
    return nc


def _prep_inputs(unalign_fb, fa, fa_parse, fb, fb_parse):
    c2 = unalign_fb.shape[1]
    c = fa.shape[1]
    mask_a = (fa_parse[0, 1:, ::4, ::4].reshape(3, HW) != 0).astype(np.float32)
    mask_b = (fb_parse[0, 1:, ::4, ::4].reshape(3, HW) != 0).astype(np.float32)
    ufb = _interp_bilinear_ac(unalign_fb[0], (64, 64)).reshape(c2, HW)

    faf = np.ascontiguousarray(fa[0].reshape(c, HW), np.float32)
    fbf = np.ascontiguousarray(fb[0].reshape(c, HW), np.float32)
    faf = faf - faf.mean(axis=1, keepdims=True, dtype=np.float32)
    fbf = fbf - fbf.mean(axis=1, keepdims=True, dtype=np.float32)
    fan = faf / np.linalg.norm(faf, axis=0, keepdims=True)
    fbn = fbf / np.linalg.norm(fbf, axis=0, keepdims=True)

    # ---- q side: group by mask_b bit-pattern, drop group-0 q's ----
    qg = (mask_b[0] > 0) * 1 + (mask_b[1] > 0) * 2 + (mask_b[2] > 0) * 4
    qlive = np.flatnonzero(qg > 0)
    qsort = qlive[np.argsort(qg[qlive], kind="stable")]
    nq_live = qsort.size
    NQ = (nq_live + 127) // 128
    qpad = NQ * 128 - nq_live

    gq = np.concatenate([qg[qsort], np.zeros(qpad, np.int64)])  # 0 = pad
    fbn_p = np.zeros((c, NQ * 128), np.float32)
    fbn_p[:, :nq_live] = fbn[:, qsort]
    ufb_q = np.zeros((NQ * 128, c2), np.float32)
    ufb_q[:nq_live] = ufb[:, qsort].T

    # per-tile schedule + M buffer [128, 65 * n_tile_groups]
    sched = []
    m_blocks = []
    for t in range(NQ):
        rows = slice(t * 128, (t + 1) * 128)
        gts = sorted(set(gq[rows][gq[rows] > 0].tolist()))
        sched.append(tuple(int(g) for g in gts))
        for g in gts:
            sel = (gq[rows] == g)
            Mt = np.zeros((128, 65), np.float32)
            Mt[sel, 0:64] = ufb_q[rows][sel]
            Mt[sel, 64] = 1.0
            m_blocks.append(Mt)
    mcomb = np.concatenate(m_blocks, axis=1).astype(ml_dtypes.bfloat16)
    MCOLS = mcomb.shape[1]

    # ---- p side: dead-column removal + per-core shards ----
    live = np.flatnonzero(mask_a.sum(axis=0) > 0)
    PC = max(456, -(-live.size // N_CORES))
    assert PC <= 512
    npad = N_CORES * PC
    perm = np.concatenate([live, np.full(npad - live.size, live[0], np.int64)])

    fbn3 = np.ascontiguousarray(
        fbn_p.reshape(2, 128, NQ * 128)).astype(ml_dtypes.bfloat16)
    fan_p = fan[:, perm].reshape(2, 128, npad).astype(ml_dtypes.bfloat16)
    in_maps = []
    for i in range(N_CORES):
        ps = slice(i * PC, (i + 1) * PC)
        in_maps.append({
            "fbn": fbn3,
            "fan": np.ascontiguousarray(fan_p[:, :, ps]),
            "mcomb": mcomb,
        })

    # host-epilogue constants
    esc = np.float32(np.exp(-CSHIFT))
    norm = np.maximum(mask_a.sum(axis=0), 1.0)
    ga = (mask_a / norm[None, :]).astype(np.float32)            # (3,HW)
    vks = (ufb @ (1.0 - mask_b).T).astype(np.float32) * esc     # (64,3)
    zk = ((1.0 - mask_b).sum(axis=1).astype(np.float32) * esc)  # (3,)

    # DMA chunk plans
    fbn_chunks = []
    rem = NQ
    first = True
    while rem > 0:
        n = min(4 if first else 5, rem)
        fbn_chunks.append(n)
        rem -= n
        first = False
    m_chunks = []
    t0 = 0
    for n in (8, 10, NQ):
        n = min(n, NQ - t0)
        if n <= 0:
            break
        m_chunks.append((t0, n))
        t0 += n

    key = (NQ, PC, tuple(sched), tuple(fbn_chunks), tuple(m_chunks), MCOLS)
    meta = (ga, vks, zk, live, perm, PC, tuple(sched))
    return in_maps, meta, (key, NQ, PC, tuple(sched), tuple(fbn_chunks),
                           tuple(m_chunks), MCOLS)


def _run(inputs, trace=False, trace_cores=None):
    unalign_fb = np.asarray(inputs["unalign_fb"], np.float32)
    fa = np.asarray(inputs["fa"], np.float32)
    fa_parse = np.asarray(inputs["fa_parse"])
    fb = np.asarray(inputs["fb"], np.float32)
    fb_parse = np.asarray(inputs["fb_parse"])

    in_maps, meta, build_args = _prep_inputs(
        unalign_fb, fa, fa_parse, fb, fb_parse)
    nc = _build_program(*build_args)
    res = run_bass_kernel_spmd(nc, in_maps, core_ids=list(range(N_CORES)),
                               trace=trace, trace_cores=trace_cores)

    ga, vks, zk, live, perm, PC, sched = meta
    c2 = unalign_fb.shape[1]
    # per core: NB p-blocks of [rows=p, 455] -> (PC, 455)
    nd_all = np.concatenate(
        [np.concatenate([res.results[i]["out_nd"][0:min(128, PC - b * 128),
                                                  b * 455:(b + 1) * 455]
                         for b in range(4)])
         for i in range(N_CORES)]).astype(np.float32)      # (8*PC, 455)
    ga_p = ga[:, perm]
    combined = np.zeros((c2, N_CORES * PC), np.float32)
    for k in range(3):
        gs = [g for g in range(1, 8) if g & (1 << k)]
        numer = sum(nd_all[:, (g - 1) * 65:(g - 1) * 65 + 64] for g in gs)
        denom = sum(nd_all[:, (g - 1) * 65 + 64] for g in gs)
        numer = numer.T + vks[:, k:k + 1]
        denom = denom + zk[k]
        combined += (ga_p[k] / denom)[None, :] * numer
    aligned = np.zeros((c2, HW), np.float32)
    aligned[:, live] = combined[:, :live.size]
    out = _interp_bilinear_ac(aligned.reshape(c2, 64, 64), (256, 256))
    return out[None], res


def kernel(**inputs):
    out, _ = _run(inputs)
    return out


# revision 11
# speedup vs baseline: 1.1626x; 1.0515x over previous
"""Trainium2 Bass kernel for LocalSemanticAlignment (sparse_attention).

Pipeline (reference semantics):
  masks   = parse[:,1:] downsampled 256->64 (nearest, stride-4)
  ufb     = bilinear-AC downsample of unalign_fb to 64x64        (host)
  fan/fbn = per-channel-centered, per-column L2-normalized fa/fb (host)
  S[q,p]  = fbn^T fan                                            (device, bf16 matmul)
  per class k: w_k = where(mask_b[q], exp(alpha*S - C), exp(-C))
  softmax over q; warped_k = ufb @ softmax; combine over k with mask_a
  output  = bilinear-AC upsample of aligned to 256x256           (host)

Key identities used on device:
 - w_k = mask_b[k,q]*exp(aS-C) + (1-mask_b[k,q])e^-C, so the e^-C part is a
   per-(k,p) constant handled on the host (vks/zk); the device only needs
   E = exp(aS - C) summed against ufb over the q's where mask_b[k,q]=1.
 - GROUP DECOMPOSITION: each q belongs to exactly one of 7 nonzero mask_b
   bit-patterns ("groups"). numer_k = sum over groups g containing k of
   numer_g, with numer_g = sum_{q in g} ufb[:,q] E[q,p] (plus the ones-col
   giving denom_g). So the numerator moving operand is 65 cols (64 ufb + 1
   ones) instead of 3*64+3=195 — 3x less PE streaming — and q's in NO group
   (~1/8) are dropped entirely (28 q-tiles instead of 32 for S + exp + DMA).

Device loop per q-tile-pair: 4 S matmuls (bf16) -> ONE exp over both tiles'
PSUM banks (halves the ACT fixed overhead; ~204ns/instr) -> per tile, per
128-p block, per group: transposed numer matmul (E stationary, M=[ufb|1]
moving, 65 cols) accumulated into per-group buckets (7*65=455 f32 cols per
p-block bank; 4 banks). Software pipeline: S(i+1) issues while exp(i) runs;
numers(i) follow and never wait (exp 1010ns < S-pair 1520ns).

PSUM: 2 S pair-buffers (2 banks each) + 4 numer banks = 8 exactly.

DMA: 3 input HWDGE queues (Sync: fbn half0, Vector: fbn half1, GpSimd:
fan + M) — each dma_start costs ~0.65us of issue time on its engine, and a
queue sustains only ~110-130GB/s, so v1's 2-queue plan supply-stalled the
PE. Scalar's queue stays clear for the exps; output rides Sync+Scalar at
the end. Warmup matmuls (HAM clock gate releases after ~3us of continuous
PE busy) target s-psum buf0, which the first real S matmul's start=True
reset anyway, so the numer-bank memsets don't wait on them.

Sharding: output columns (p) split across 8 cores; each core holds full
fbn/M (keys/values) and computes its shard end-to-end. No collectives.
Numerator/denominator buckets ship raw (bf16) and the final
divide+combine (trivial) happens on the host.
"""

import numpy as np
import ml_dtypes

import concourse.bass as bass
import concourse.bacc as bacc
import concourse.mybir as mybir
from concourse import tile
from concourse.bass_utils import run_bass_kernel_spmd

ALPHA = 100.0
# global logit shift: exp(alpha*S - CSHIFT); softmax-shift-exact, the "+1"
# weights of masked-out q's are scaled by exp(-CSHIFT) on the host (vks/zk).
# 30 (not 60): E is the STATIONARY numer-matmul operand in bf16; at 60 a
# column whose max logit sits far below the shift had its tiny weights
# vanish in the PE weight path. At 30 the max logit (~90) keeps
# exp(90-30)=e60 inside bf16/f32 range.
CSHIFT = 30.0
N_CORES = 8
HW = 4096
NWARM = 28   # PE warm-up matmuls (~107ns each at the pre-ramp clock):
             # bridge the first-chunk DMA wait AND give the HAM clock gate
             # its continuous-busy window so real matmuls run at 2.4GHz.

F32 = mybir.dt.float32
BF16 = mybir.dt.bfloat16
EXP = mybir.ActivationFunctionType.Exp


def _interp_bilinear_ac(x, size):
    """torch F.interpolate bilinear align_corners=True; x: (C,H,W) float32."""
    x = np.ascontiguousarray(x, np.float32)
    H, W = x.shape[-2], x.shape[-1]
    h, w = size

    def coords(n_out, n_in):
        if n_out == 1:
            return np.zeros((1,), np.float32)
        return np.arange(n_out, dtype=np.float32) * np.float32((n_in - 1) / (n_out - 1))

    ry, rx = coords(h, H), coords(w, W)
    y0 = np.floor(ry).astype(np.int32)
    x0 = np.floor(rx).astype(np.int32)
    y1 = np.clip(y0 + 1, 0, H - 1)
    x1 = np.clip(x0 + 1, 0, W - 1)
    wy = (ry - y0.astype(np.float32))[None, :, None]
    wx = (rx - x0.astype(np.float32))[None, None, :]
    rows = x[:, y0, :] * (1.0 - wy) + x[:, y1, :] * wy
    return (rows[:, :, x0] * (1.0 - wx) + rows[:, :, x1] * wx).astype(np.float32)


_NC_CACHE = {}


def _build_program(key, NQ, PC, sched, fbn_chunks, m_chunks, MCOLS):
    """sched: tuple per tile of (groups tuple); groups are 1..7.
    fbn_chunks: tile counts per fbn DMA chunk. m_chunks: (tile0, ntiles)
    chunking of the M buffer (column offsets derived from sched)."""
    if key in _NC_CACHE:
        return _NC_CACHE[key]

    nc = bacc.Bacc("TRN2", target_bir_lowering=False, debug=False,
                   num_devices=N_CORES)

    fbn_d = nc.dram_tensor("fbn", [2, 128, NQ * 128], BF16,
                           kind="ExternalInput").ap()
    fan_d = nc.dram_tensor("fan", [2, 128, PC], BF16, kind="ExternalInput").ap()
    m_d = nc.dram_tensor("mcomb", [128, MCOLS], BF16, kind="ExternalInput").ap()
    out_d = nc.dram_tensor("out_nd", [128, 7 * 260], BF16,
                           kind="ExternalOutput").ap()

    NB = (PC + 127) // 128          # p blocks (4 for PC<=512)
    NPAIR = (NQ + 1) // 2

    # last tile touching each group -> ship its buckets mid-stream
    tlast = {}
    for t, groups in enumerate(sched):
        for g in groups:
            tlast[g] = t

    # M column offset per (tile, group)
    moff = {}
    c = 0
    for t, groups in enumerate(sched):
        for g in groups:
            moff[(t, g)] = c
            c += 65
    assert c == MCOLS

    # per-tile M column ranges (for chunked DMA)
    tile_m0 = []
    c = 0
    for t, groups in enumerate(sched):
        tile_m0.append(c)
        c += 65 * len(groups)
    tile_m0.append(c)

    with tile.TileContext(nc) as tc:
        with (
            tc.tile_pool(name="io", bufs=1) as io,
            tc.tile_pool(name="big", bufs=1) as big,
            tc.tile_pool(name="expp", bufs=3) as expp,
            tc.tile_pool(name="spsum", bufs=2, space="PSUM") as spsum,
            tc.tile_pool(name="npsum", bufs=1, space="PSUM") as npsum,
            tc.tile_pool(name="fin", bufs=1) as fin,
        ):
            # numer buckets: one bank per 128-p block; cols g*65..g*65+65
            # hold group g's [64 ufb numer | 1 denom] for that block's p rows
            nb = [npsum.tile([128, 512], F32, tag=f"nb{b}", name=f"nb{b}")
                  for b in range(NB)]

            wz_sb = io.tile([128, 128], BF16, tag="wz")
            nc.vector.memset(wz_sb[:], 0.0)
            cb_sb = io.tile([128, 1], F32, tag="cb")
            nc.vector.memset(cb_sb[:], -CSHIFT)

            # S psum pair-buffers allocated BEFORE warmups so the warmup
            # scribbles land in buf0 (cleared by s_mms(0)'s start=True).
            s_ps = {}
            s_ps[0] = spsum.tile([128, 1024], F32, tag="s", name="s_0")

            for _ in range(NWARM):
                nc.tensor.matmul(s_ps[0][:, 0:128], wz_sb[:], wz_sb[:],
                                 start=True, stop=True)

            # zero the numer accumulator banks (matmuls accumulate with
            # start=False throughout; start=True would clear a whole bank
            # and wipe sibling groups). GPSIMD cannot touch PSUM -> DVE.
            for b in range(NB):
                nc.vector.memset(nb[b][:], 0.0)

            fan_sb = [io.tile([128, PC], BF16, tag=f"fan{c2}",
                              name=f"fan_sb{c2}") for c2 in range(2)]
            fbn_sb = [big.tile([128, NQ * 128], BF16, tag=f"fbn{c2}",
                               name=f"fbn_sb{c2}") for c2 in range(2)]
            m_sb = big.tile([128, MCOLS], BF16, tag="mcomb")

            # --- DMA issue plan: the two HWDGE queues (SP + ACT) ---
            # Sync: interleaved fbn half chunks (idle engine, ~0.65us issue
            # each), small first chunks so S(0) starts early; mid-stream it
            # also ships finished group buckets. Scalar: fan halves, M c0,
            # the exp-table preload, remaining M chunks — all before its
            # exp stream begins.
            def fbn_dma(eng, half, ci):
                q0 = sum(fbn_chunks[:ci])
                qs = slice(q0 * 128, (q0 + fbn_chunks[ci]) * 128)
                eng.dma_start(fbn_sb[half][:, qs], fbn_d[half][:, qs])

            def m_dma(eng, mi):
                t0, nt = m_chunks[mi]
                ms = slice(tile_m0[t0], tile_m0[min(t0 + nt, NQ)])
                eng.dma_start(m_sb[:, ms], m_d[:, ms])

            for ci in range(len(fbn_chunks)):
                fbn_dma(nc.sync, 0, ci)
                fbn_dma(nc.sync, 1, ci)
            nc.scalar.dma_start(fan_sb[0][:], fan_d[0])
            nc.scalar.dma_start(fan_sb[1][:], fan_d[1])
            m_dma(nc.scalar, 0)
            dum_sb = io.tile([128, 1], BF16, tag="dum")
            nc.scalar.activation(dum_sb[:], cb_sb[:], EXP, scale=1.0)
            for mi in range(1, len(m_chunks)):
                m_dma(nc.scalar, mi)

            def s_mms(i, sp):
                for h in range(2):
                    t = 2 * i + h
                    if t >= NQ:
                        return
                    qs = slice(t * 128, (t + 1) * 128)
                    hs = slice(h * 512, h * 512 + PC)
                    nc.tensor.matmul(sp[:, hs], fbn_sb[0][:, qs],
                                     fan_sb[0][:], start=True, stop=False)
                    nc.tensor.matmul(sp[:, hs], fbn_sb[1][:, qs],
                                     fan_sb[1][:], start=False, stop=True)

            def numers(i, e_sb):
                for h in range(2):
                    t = 2 * i + h
                    if t >= NQ:
                        return
                    for b in range(NB):
                        bw = min(128, PC - b * 128)
                        eb = e_sb[:, h * 512 + b * 128:h * 512 + b * 128 + bw]
                        for g in sched[t]:
                            last = (t == NQ - 1 and g == sched[t][-1])
                            mc = moff[(t, g)]
                            nc.tensor.matmul(
                                nb[b][0:bw, (g - 1) * 65:(g - 1) * 65 + 65],
                                eb, m_sb[:, mc:mc + 65],
                                start=False, stop=last)

            # group-major output staging: cols (g-1)*260 + b*65. Each
            # group's buckets are copied (DVE, idle) and DMA'd (Sync) as
            # soon as its last q-tile's matmuls retire — only the final
            # group drains after the stream.
            o_sb = fin.tile([128, 7 * 260], BF16, tag="o")

            def ship_group(g):
                o0 = (g - 1) * 260
                for b in range(NB):
                    nc.vector.tensor_copy(
                        o_sb[:, o0 + b * 65:o0 + (b + 1) * 65],
                        nb[b][:, (g - 1) * 65:(g - 1) * 65 + 65])
                nc.sync.dma_start(out_d[:, o0:o0 + 260],
                                  o_sb[:, o0:o0 + 260])

            # software pipeline: S(i+1) issues while exp(i) is in flight;
            # numers(i) read e(i) afterwards and never stall the PE.
            s_mms(0, s_ps[0])
            for i in range(NPAIR):
                if i + 1 < NPAIR:
                    s_ps[i + 1] = spsum.tile([128, 1024], F32, tag="s",
                                             name=f"s_{i + 1}")
                    s_mms(i + 1, s_ps[i + 1])
                sp = s_ps.pop(i)
                # one exp across the pair's two banks (cols between PC and
                # 512 were zeroed by the S start=True and exp to e^-30,
                # never read)
                W = 512 + PC if 2 * i + 1 < NQ else PC
                e_sb = expp.tile([128, 1024], BF16, tag="e")
                nc.scalar.activation(e_sb[:, 0:W], sp[:, 0:W], EXP,
                                     scale=ALPHA, bias=cb_sb[:])
                numers(i, e_sb)
                for g in range(1, 8):
                    if g in tlast and tlast[g] // 2 == i:
                        ship_group(g)

    nc.compile()
    _NC_CACHE[key] = nc
    return nc


def _prep_inputs(unalign_fb, fa, fa_parse, fb, fb_parse):
    c2 = unalign_fb.shape[1]
    c = fa.shape[1]
    mask_a = (fa_parse[0, 1:, ::4, ::4].reshape(3, HW) != 0).astype(np.float32)
    mask_b = (fb_parse[0, 1:, ::4, ::4].reshape(3, HW) != 0).astype(np.float32)
    ufb = _interp_bilinear_ac(unalign_fb[0], (64, 64)).reshape(c2, HW)

    faf = np.ascontiguousarray(fa[0].reshape(c, HW), np.float32)
    fbf = np.ascontiguousarray(fb[0].reshape(c, HW), np.float32)
    faf = faf - faf.mean(axis=1, keepdims=True, dtype=np.float32)
    fbf = fbf - fbf.mean(axis=1, keepdims=True, dtype=np.float32)
    fan = faf / np.linalg.norm(faf, axis=0, keepdims=True)
    fbn = fbf / np.linalg.norm(fbf, axis=0, keepdims=True)

    # ---- q side: group by mask_b bit-pattern, drop group-0 q's ----
    qg = (mask_b[0] > 0) * 1 + (mask_b[1] > 0) * 2 + (mask_b[2] > 0) * 4
    qlive = np.flatnonzero(qg > 0)
    qsort = qlive[np.argsort(qg[qlive], kind="stable")]
    nq_live = qsort.size
    NQ = (nq_live + 127) // 128
    qpad = NQ * 128 - nq_live

    gq = np.concatenate([qg[qsort], np.zeros(qpad, np.int64)])  # 0 = pad
    fbn_p = np.zeros((c, NQ * 128), np.float32)
    fbn_p[:, :nq_live] = fbn[:, qsort]
    ufb_q = np.zeros((NQ * 128, c2), np.float32)
    ufb_q[:nq_live] = ufb[:, qsort].T

    # per-tile schedule + M buffer [128, 65 * n_tile_groups]
    sched = []
    m_blocks = []
    for t in range(NQ):
        rows = slice(t * 128, (t + 1) * 128)
        gts = sorted(set(gq[rows][gq[rows] > 0].tolist()))
        sched.append(tuple(int(g) for g in gts))
        for g in gts:
            sel = (gq[rows] == g)
            Mt = np.zeros((128, 65), np.float32)
            Mt[sel, 0:64] = ufb_q[rows][sel]
            Mt[sel, 64] = 1.0
            m_blocks.append(Mt)
    mcomb = np.concatenate(m_blocks, axis=1).astype(ml_dtypes.bfloat16)
    MCOLS = mcomb.shape[1]

    # ---- p side: dead-column removal + per-core shards ----
    live = np.flatnonzero(mask_a.sum(axis=0) > 0)
    PC = max(456, -(-live.size // N_CORES))
    assert PC <= 512
    npad = N_CORES * PC
    perm = np.concatenate([live, np.full(npad - live.size, live[0], np.int64)])

    fbn3 = np.ascontiguousarray(
        fbn_p.reshape(2, 128, NQ * 128)).astype(ml_dtypes.bfloat16)
    fan_p = fan[:, perm].reshape(2, 128, npad).astype(ml_dtypes.bfloat16)
    in_maps = []
    for i in range(N_CORES):
        ps = slice(i * PC, (i + 1) * PC)
        in_maps.append({
            "fbn": fbn3,
            "fan": np.ascontiguousarray(fan_p[:, :, ps]),
            "mcomb": mcomb,
        })

    # host-epilogue constants
    esc = np.float32(np.exp(-CSHIFT))
    norm = np.maximum(mask_a.sum(axis=0), 1.0)
    ga = (mask_a / norm[None, :]).astype(np.float32)            # (3,HW)
    vks = (ufb @ (1.0 - mask_b).T).astype(np.float32) * esc     # (64,3)
    zk = ((1.0 - mask_b).sum(axis=1).astype(np.float32) * esc)  # (3,)

    # DMA chunk plans: small leading fbn chunks so S(0) starts early
    fbn_chunks = []
    rem = NQ
    plan = [2, 3, 4]
    while rem > 0:
        n = min(plan[0] if plan else 5, rem)
        if plan:
            plan.pop(0)
        fbn_chunks.append(n)
        rem -= n
    m_chunks = []
    t0 = 0
    for n in (8, 10, NQ):
        n = min(n, NQ - t0)
        if n <= 0:
            break
        m_chunks.append((t0, n))
        t0 += n

    key = (NQ, PC, tuple(sched), tuple(fbn_chunks), tuple(m_chunks), MCOLS)
    meta = (ga, vks, zk, live, perm, PC, tuple(sched))
    return in_maps, meta, (key, NQ, PC, tuple(sched), tuple(fbn_chunks),
                           tuple(m_chunks), MCOLS)


def _run(inputs, trace=False, trace_cores=None):
    unalign_fb = np.asarray(inputs["unalign_fb"], np.float32)
    fa = np.asarray(inputs["fa"], np.float32)
    fa_parse = np.asarray(inputs["fa_parse"])
    fb = np.asarray(inputs["fb"], np.float32)
    fb_parse = np.asarray(inputs["fb_parse"])

    in_maps, meta, build_args = _prep_inputs(
        unalign_fb, fa, fa_parse, fb, fb_parse)
    nc = _build_program(*build_args)
    res = run_bass_kernel_spmd(nc, in_maps, core_ids=list(range(N_CORES)),
                               trace=trace, trace_cores=trace_cores)

    ga, vks, zk, live, perm, PC, sched = meta
    c2 = unalign_fb.shape[1]
    # per core: group-major [128, 7*260] -> per group (PC, 65)
    nd_all = np.zeros((7, N_CORES * PC, 65), np.float32)
    for i in range(N_CORES):
        o = np.asarray(res.results[i]["out_nd"]).astype(np.float32)
        arr = o.reshape(128, 7, 4, 65)
        for g in range(7):
            nd_all[g, i * PC:(i + 1) * PC] = np.concatenate(
                [arr[0:min(128, PC - b * 128), g, b, :] for b in range(4)])
    ga_p = ga[:, perm]
    combined = np.zeros((c2, N_CORES * PC), np.float32)
    for k in range(3):
        gs = [g for g in range(1, 8) if g & (1 << k)]
        numer = sum(nd_all[g - 1][:, 0:64] for g in gs)
        denom = sum(nd_all[g - 1][:, 64] for g in gs)
        numer = numer.T + vks[:, k:k + 1]
        denom = denom + zk[k]
        combined += (ga_p[k] / denom)[None, :] * numer
    aligned = np.zeros((c2, HW), np.float32)
    aligned[:, live] = combined[:, :live.size]
    out = _interp_bilinear_ac(aligned.reshape(c2, 64, 64), (256, 256))
    return out[None], res


def kernel(**inputs):
    out, _ = _run(inputs)
    return out


# revision 12
# speedup vs baseline: 1.1942x; 1.0272x over previous
"""Trainium2 Bass kernel for LocalSemanticAlignment (sparse_attention).

Pipeline (reference semantics):
  masks   = parse[:,1:] downsampled 256->64 (nearest, stride-4)
  ufb     = bilinear-AC downsample of unalign_fb to 64x64        (host)
  fan/fbn = per-channel-centered, per-column L2-normalized fa/fb (host)
  S[q,p]  = fbn^T fan                                            (device, bf16 matmul)
  per class k: w_k = where(mask_b[q], exp(alpha*S - C), exp(-C))
  softmax over q; warped_k = ufb @ softmax; combine over k with mask_a
  output  = bilinear-AC upsample of aligned to 256x256           (host)

Key identities used on device:
 - w_k = mask_b[k,q]*exp(aS-C) + (1-mask_b[k,q])e^-C: the e^-C part is a
   per-(k,p) host constant (vks/zk); the device only needs E = exp(aS - C)
   summed against ufb over q's with mask_b[k,q]=1.
 - GROUP DECOMPOSITION: each q belongs to exactly one of 7 nonzero mask_b
   bit-patterns ("groups"); each p to one of 7 mask_a patterns ("stripes").
   numer_k = sum over groups g containing k of numer_g, where numer_g =
   sum_{q in g} ufb[:,q] E[q,p] plus a ones-col giving denom_g. The numer
   moving operand is 65 cols (vs 3*64+3) and group-0 q's drop out entirely
   (29 live q-tiles, not 32).
 - COMPAT SPARSITY: E[q,p] is only consumed when q's group and p's stripe
   share a class. With stripes ordered [ab,a,ac,abc,c,bc,b], each group's
   compatible stripe set has a contiguous COVER; S matmuls and the exp
   only span the cover (~15-25% less PE + ACT work), and numer matmuls
   skip 128-p blocks fully outside a group's exact compat set. Values
   outside the cover are psum zeros (start=True clears the whole bank) ->
   exp(e^-30); e_sb cols outside the exp cover hold stale-but-finite bf16
   from 3 pairs earlier (first 3 pairs exp full width to initialize the
   ring) and only ever flow into bucket columns the host never reads.

Device loop per q-tile-pair: 4 cover-S matmuls (bf16) -> ONE exp across
both tiles' PSUM banks (pair-exp halves the ~260ns ACT fixed overhead) ->
per tile, per 128-p block, per group: transposed numer matmul (E
stationary, M=[ufb|1] moving, 65 cols) into per-group bucket columns
(7*65=455 f32 per bank; 4 banks). Software pipeline: S(i+1) issues while
exp(i) runs; numers(i) follow. PSUM: 2 S pair-buffers + 4 numer banks = 8.

DMA (both HWDGE queues; issues are completion-chained with 2 outstanding
per queue, and cold-start latency is ~2.5-3us, so few LARGE chunks):
Sync: interleaved fbn half-chunks, then finished group buckets mid-stream.
Scalar: fan halves, M chunks, exp-table preload — all before its exp
stream. Warmups (HAM clock gate wants ~3us of continuous PE busy) target
s-psum buf0 which the first real S matmul clears anyway.

Sharding: output columns (p) split across 8 cores (p-stripes striped
uniformly across cores so every core has identical structure); each core
holds full fbn/M and computes its shard end-to-end. No collectives.
Buckets ship raw (bf16, group-major) as each group's accumulation closes;
the trivial divide+combine happens on the host.
"""

import numpy as np
import ml_dtypes

import concourse.bass as bass
import concourse.bacc as bacc
import concourse.mybir as mybir
from concourse import tile
from concourse.bass_utils import run_bass_kernel_spmd

ALPHA = 100.0
# global logit shift: exp(alpha*S - CSHIFT); softmax-shift-exact, the "+1"
# weights of masked-out q's are scaled by exp(-CSHIFT) on the host (vks/zk).
# 30 (not 60): E is the STATIONARY numer-matmul operand in bf16; at 60 a
# column whose max logit sits far below the shift had its tiny weights
# vanish in the PE weight path. At 30 the max logit (~90) keeps
# exp(90-30)=e60 inside bf16/f32 range.
CSHIFT = 30.0
N_CORES = 8
HW = 4096
NWARM = 34   # PE warm-up matmuls (~107ns each at the pre-ramp clock):
             # bridge the cold first-chunk DMA latency AND give the HAM
             # clock gate its ~3us continuous-busy window.

# p-stripe order (bitmask a=1,b=2,c=4): [ab, a, ac, abc, c, bc, b] makes
# every q-group's compatible stripe set contiguous-or-nearly (cover).
STRIPE_ORDER = (3, 1, 5, 7, 4, 6, 2)

F32 = mybir.dt.float32
BF16 = mybir.dt.bfloat16
EXP = mybir.ActivationFunctionType.Exp


def _interp_bilinear_ac(x, size):
    """torch F.interpolate bilinear align_corners=True; x: (C,H,W) float32."""
    x = np.ascontiguousarray(x, np.float32)
    H, W = x.shape[-2], x.shape[-1]
    h, w = size

    def coords(n_out, n_in):
        if n_out == 1:
            return np.zeros((1,), np.float32)
        return np.arange(n_out, dtype=np.float32) * np.float32((n_in - 1) / (n_out - 1))

    ry, rx = coords(h, H), coords(w, W)
    y0 = np.floor(ry).astype(np.int32)
    x0 = np.floor(rx).astype(np.int32)
    y1 = np.clip(y0 + 1, 0, H - 1)
    x1 = np.clip(x0 + 1, 0, W - 1)
    wy = (ry - y0.astype(np.float32))[None, :, None]
    wx = (rx - x0.astype(np.float32))[None, None, :]
    rows = x[:, y0, :] * (1.0 - wy) + x[:, y1, :] * wy
    return (rows[:, :, x0] * (1.0 - wx) + rows[:, :, x1] * wx).astype(np.float32)


def _covers(sched, sbounds, PC):
    """Per-tile S/exp column cover [a,b) and per-(tile,group) compat col
    ranges, from the stripe bounds sbounds[h]=(c0,c1) keyed by bitmask."""
    tile_cover = []
    for groups in sched:
        cs = [sbounds[h] for h in STRIPE_ORDER
              if any(h & g for g in groups)]
        a = min(c[0] for c in cs)
        b = max(c[1] for c in cs)
        tile_cover.append((a, min(b, PC)))
    return tile_cover


def _compat_blocks(g, sbounds, NB, PC):
    """Set of 128-p block indices overlapping group g's exact compat set."""
    out = set()
    for h in STRIPE_ORDER:
        if h & g:
            c0, c1 = sbounds[h]
            for b in range(NB):
                b0, b1 = b * 128, min(b * 128 + 128, PC)
                if c0 < b1 and c1 > b0:
                    out.add(b)
    return out


_NC_CACHE = {}


def _build_program(key):
    if key in _NC_CACHE:
        return _NC_CACHE[key]
    (NQ, PC, sched, fbn_chunks, m_chunks, MCOLS, sbounds_t) = key
    sched = [tuple(s) for s in sched]
    sbounds = dict(sbounds_t)

    nc = bacc.Bacc("TRN2", target_bir_lowering=False, debug=False,
                   num_devices=N_CORES)

    fbn_d = nc.dram_tensor("fbn", [2, 128, NQ * 128], BF16,
                           kind="ExternalInput").ap()
    fan_d = nc.dram_tensor("fan", [2, 128, PC], BF16, kind="ExternalInput").ap()
    m_d = nc.dram_tensor("mcomb", [128, MCOLS], BF16, kind="ExternalInput").ap()
    out_d = nc.dram_tensor("out_nd", [128, 7 * 260], BF16,
                           kind="ExternalOutput").ap()

    NB = (PC + 127) // 128
    assert NB == 4
    NPAIR = (NQ + 1) // 2
    tile_cover = _covers(sched, sbounds, PC)
    cblocks = {g: _compat_blocks(g, sbounds, NB, PC) for g in range(1, 8)}

    # M column offset per (tile, group)
    moff = {}
    c = 0
    for t, groups in enumerate(sched):
        for g in groups:
            moff[(t, g)] = c
            c += 65
    assert c == MCOLS
    tile_m0 = []
    c = 0
    for t, groups in enumerate(sched):
        tile_m0.append(c)
        c += 65 * len(groups)
    tile_m0.append(c)

    # last executed numer matmul per bank (for the stop flag) and last
    # tile per group (for mid-stream bucket shipping)
    lastb = {}
    tlast = {}
    for t, groups in enumerate(sched):
        for g in groups:
            tlast[g] = t
            for b in range(NB):
                if b in cblocks[g]:
                    lastb[b] = (t, g)

    with tile.TileContext(nc) as tc:
        with (
            tc.tile_pool(name="io", bufs=1) as io,
            tc.tile_pool(name="big", bufs=1) as big,
            tc.tile_pool(name="expp", bufs=3) as expp,
            tc.tile_pool(name="spsum", bufs=2, space="PSUM") as spsum,
            tc.tile_pool(name="npsum", bufs=1, space="PSUM") as npsum,
            tc.tile_pool(name="fin", bufs=1) as fin,
        ):
            nb = [npsum.tile([128, 512], F32, tag=f"nb{b}", name=f"nb{b}")
                  for b in range(NB)]

            wz_sb = io.tile([128, 128], BF16, tag="wz")
            nc.vector.memset(wz_sb[:], 0.0)
            cb_sb = io.tile([128, 1], F32, tag="cb")
            nc.vector.memset(cb_sb[:], -CSHIFT)

            s_ps = {}
            s_ps[0] = spsum.tile([128, 1024], F32, tag="s", name="s_0")

            for _ in range(NWARM):
                nc.tensor.matmul(s_ps[0][:, 0:128], wz_sb[:], wz_sb[:],
                                 start=True, stop=True)

            # numer banks accumulate with start=False throughout
            # (start=True would clear a whole bank, wiping sibling groups)
            for b in range(NB):
                nc.vector.memset(nb[b][:], 0.0)

            fan_sb = [io.tile([128, PC], BF16, tag=f"fan{c2}",
                              name=f"fan_sb{c2}") for c2 in range(2)]
            fbn_sb = [big.tile([128, NQ * 128], BF16, tag=f"fbn{c2}",
                               name=f"fbn_sb{c2}") for c2 in range(2)]
            m_sb = big.tile([128, MCOLS], BF16, tag="mcomb")

            def fbn_dma(eng, half, ci):
                q0 = sum(fbn_chunks[:ci])
                qs = slice(q0 * 128, (q0 + fbn_chunks[ci]) * 128)
                eng.dma_start(fbn_sb[half][:, qs], fbn_d[half][:, qs])

            def m_dma(eng, mi):
                t0, nt = m_chunks[mi]
                ms = slice(tile_m0[t0], tile_m0[min(t0 + nt, NQ)])
                eng.dma_start(m_sb[:, ms], m_d[:, ms])

            for ci in range(len(fbn_chunks)):
                fbn_dma(nc.sync, 0, ci)
                fbn_dma(nc.sync, 1, ci)
            nc.scalar.dma_start(fan_sb[0][:], fan_d[0])
            nc.scalar.dma_start(fan_sb[1][:], fan_d[1])
            m_dma(nc.scalar, 0)
            dum_sb = io.tile([128, 1], BF16, tag="dum")
            nc.scalar.activation(dum_sb[:], cb_sb[:], EXP, scale=1.0)
            for mi in range(1, len(m_chunks)):
                m_dma(nc.scalar, mi)

            def s_mms(i, sp):
                for h in range(2):
                    t = 2 * i + h
                    if t >= NQ:
                        return
                    a, b = tile_cover[t]
                    qs = slice(t * 128, (t + 1) * 128)
                    hs = slice(h * 512 + a, h * 512 + b)
                    nc.tensor.matmul(sp[:, hs], fbn_sb[0][:, qs],
                                     fan_sb[0][:, a:b], start=True, stop=False)
                    nc.tensor.matmul(sp[:, hs], fbn_sb[1][:, qs],
                                     fan_sb[1][:, a:b], start=False, stop=True)

            def numers(i, e_sb):
                for h in range(2):
                    t = 2 * i + h
                    if t >= NQ:
                        return
                    for b in range(NB):
                        bw = min(128, PC - b * 128)
                        eb = e_sb[:, h * 512 + b * 128:h * 512 + b * 128 + bw]
                        for g in sched[t]:
                            if b not in cblocks[g]:
                                continue
                            last = (lastb[b] == (t, g))
                            mc = moff[(t, g)]
                            nc.tensor.matmul(
                                nb[b][0:bw, (g - 1) * 65:(g - 1) * 65 + 65],
                                eb, m_sb[:, mc:mc + 65],
                                start=False, stop=last)

            # group-major output staging: cols (g-1)*260 + b*65; each
            # group ships as soon as its last q-tile's matmuls retire.
            # The final pair's groups split copies across DVE+ACT and
            # DMA on Scalar (both idle by then); earlier groups use the
            # otherwise-idle DVE + Sync.
            o_sb = fin.tile([128, 7 * 260], BF16, tag="o")

            def ship_group(g, at_end):
                o0 = (g - 1) * 260
                for b in range(NB):
                    eng = nc.scalar if (at_end and b % 2 == 1) else nc.vector
                    src = nb[b][:, (g - 1) * 65:(g - 1) * 65 + 65]
                    dst = o_sb[:, o0 + b * 65:o0 + (b + 1) * 65]
                    if eng is nc.scalar:
                        nc.scalar.copy(dst, src)
                    else:
                        nc.vector.tensor_copy(dst, src)
                eng = nc.scalar if at_end else nc.sync
                eng.dma_start(out_d[:, o0:o0 + 260], o_sb[:, o0:o0 + 260])

            # software pipeline: S(i+1) issues while exp(i) is in flight;
            # numers(i) read e(i) afterwards.
            s_mms(0, s_ps[0])
            for i in range(NPAIR):
                if i + 1 < NPAIR:
                    s_ps[i + 1] = spsum.tile([128, 1024], F32, tag="s",
                                             name=f"s_{i + 1}")
                    s_mms(i + 1, s_ps[i + 1])
                sp = s_ps.pop(i)
                # one exp across the pair's two banks over the cover
                # union; cols outside any S cover are psum zeros ->
                # e^-30. First 3 pairs run full width to initialize the
                # e ring buffers (later cover-skipped cols read stale
                # values that only reach host-ignored bucket columns).
                t0, t1 = 2 * i, min(2 * i + 1, NQ - 1)
                if i < 3:
                    a0, b1 = 0, PC
                else:
                    a0 = tile_cover[t0][0]
                    b1 = tile_cover[t1][1]
                W0 = a0
                W1 = 512 + b1 if t1 > t0 else b1
                e_sb = expp.tile([128, 1024], BF16, tag="e")
                nc.scalar.activation(e_sb[:, W0:W1], sp[:, W0:W1], EXP,
                                     scale=ALPHA, bias=cb_sb[:])
                numers(i, e_sb)
                for g in range(1, 8):
                    if g in tlast and tlast[g] // 2 == i:
                        ship_group(g, i == NPAIR - 1)

    nc.compile()
    _NC_CACHE[key] = nc
    return nc


def _prep_inputs(unalign_fb, fa, fa_parse, fb, fb_parse):
    c2 = unalign_fb.shape[1]
    c = fa.shape[1]
    mask_a = (fa_parse[0, 1:, ::4, ::4].reshape(3, HW) != 0).astype(np.float32)
    mask_b = (fb_parse[0, 1:, ::4, ::4].reshape(3, HW) != 0).astype(np.float32)
    ufb = _interp_bilinear_ac(unalign_fb[0], (64, 64)).reshape(c2, HW)

    faf = np.ascontiguousarray(fa[0].reshape(c, HW), np.float32)
    fbf = np.ascontiguousarray(fb[0].reshape(c, HW), np.float32)
    faf = faf - faf.mean(axis=1, keepdims=True, dtype=np.float32)
    fbf = fbf - fbf.mean(axis=1, keepdims=True, dtype=np.float32)
    fan = faf / np.linalg.norm(faf, axis=0, keepdims=True)
    fbn = fbf / np.linalg.norm(fbf, axis=0, keepdims=True)

    # ---- q side: group by mask_b bit-pattern, drop group-0 q's ----
    qg = ((mask_b[0] > 0) * 1 + (mask_b[1] > 0) * 2
          + (mask_b[2] > 0) * 4).astype(np.int64)
    qlive = np.flatnonzero(qg > 0)
    qsort = qlive[np.argsort(qg[qlive], kind="stable")]
    nq_live = qsort.size
    NQ = (nq_live + 127) // 128
    qpad = NQ * 128 - nq_live

    gq = np.concatenate([qg[qsort], np.zeros(qpad, np.int64)])  # 0 = pad
    fbn_p = np.zeros((c, NQ * 128), np.float32)
    fbn_p[:, :nq_live] = fbn[:, qsort]
    ufb_q = np.zeros((NQ * 128, c2), np.float32)
    ufb_q[:nq_live] = ufb[:, qsort].T

    sched = []
    m_blocks = []
    for t in range(NQ):
        rows = slice(t * 128, (t + 1) * 128)
        gts = sorted(set(gq[rows][gq[rows] > 0].tolist()))
        sched.append(tuple(int(g) for g in gts))
        for g in gts:
            sel = (gq[rows] == g)
            Mt = np.zeros((128, 65), np.float32)
            Mt[sel, 0:64] = ufb_q[rows][sel]
            Mt[sel, 64] = 1.0
            m_blocks.append(Mt)
    mcomb = np.concatenate(m_blocks, axis=1).astype(ml_dtypes.bfloat16)
    MCOLS = mcomb.shape[1]

    # ---- p side: 7 mask_a-pattern stripes, striped uniformly over cores
    pg = ((mask_a[0] > 0) * 1 + (mask_a[1] > 0) * 2
          + (mask_a[2] > 0) * 4).astype(np.int64)
    stripe_p = {}
    wh = {}
    for h in STRIPE_ORDER:
        ps = np.flatnonzero(pg == h)
        stripe_p[h] = ps
        wh[h] = max(1, -(-ps.size // N_CORES)) if ps.size else 0
    PC = sum(wh.values())
    assert 385 <= PC <= 512, PC
    sbounds = {}
    c0 = 0
    for h in STRIPE_ORDER:
        sbounds[h] = (c0, c0 + wh[h])
        c0 += wh[h]

    perm_cols = []
    for i in range(N_CORES):
        for h in STRIPE_ORDER:
            ps = stripe_p[h]
            if ps.size == 0:
                continue
            seg = ps[i * wh[h]:(i + 1) * wh[h]]
            if seg.size < wh[h]:
                seg = np.concatenate(
                    [seg, np.full(wh[h] - seg.size, ps[-1], np.int64)])
            perm_cols.append(seg)
    perm = np.concatenate(perm_cols)          # (N_CORES*PC,)

    fbn3 = np.ascontiguousarray(
        fbn_p.reshape(2, 128, NQ * 128)).astype(ml_dtypes.bfloat16)
    fan_p = fan[:, perm].reshape(2, 128, N_CORES * PC).astype(ml_dtypes.bfloat16)
    in_maps = []
    for i in range(N_CORES):
        ps = slice(i * PC, (i + 1) * PC)
        in_maps.append({
            "fbn": fbn3,
            "fan": np.ascontiguousarray(fan_p[:, :, ps]),
            "mcomb": mcomb,
        })

    # host-epilogue constants
    esc = np.float32(np.exp(-CSHIFT))
    norm = np.maximum(mask_a.sum(axis=0), 1.0)
    ga = (mask_a / norm[None, :]).astype(np.float32)            # (3,HW)
    vks = (ufb @ (1.0 - mask_b).T).astype(np.float32) * esc     # (64,3)
    zk = ((1.0 - mask_b).sum(axis=1).astype(np.float32) * esc)  # (3,)

    # DMA chunk plans: few LARGE chunks (issues are completion-chained
    # with 2 outstanding per queue; cold-start latency ~2.5-3us)
    fbn_chunks = []
    rem = NQ
    plan = [3, 5, 10]
    while rem > 0:
        n = min(plan.pop(0) if plan else 11, rem)
        fbn_chunks.append(n)
        rem -= n
    m_chunks = []
    t0 = 0
    for n in (10, NQ):
        n = min(n, NQ - t0)
        if n <= 0:
            break
        m_chunks.append((t0, n))
        t0 += n

    sbounds_t = tuple(sorted(sbounds.items()))
    key = (NQ, PC, tuple(sched), tuple(fbn_chunks), tuple(m_chunks),
           MCOLS, sbounds_t)
    meta = (ga, vks, zk, perm, PC)
    return in_maps, meta, key


def _run(inputs, trace=False, trace_cores=None):
    unalign_fb = np.asarray(inputs["unalign_fb"], np.float32)
    fa = np.asarray(inputs["fa"], np.float32)
    fa_parse = np.asarray(inputs["fa_parse"])
    fb = np.asarray(inputs["fb"], np.float32)
    fb_parse = np.asarray(inputs["fb_parse"])

    in_maps, meta, key = _prep_inputs(
        unalign_fb, fa, fa_parse, fb, fb_parse)
    nc = _build_program(key)
    res = run_bass_kernel_spmd(nc, in_maps, core_ids=list(range(N_CORES)),
                               trace=trace, trace_cores=trace_cores)

    ga, vks, zk, perm, PC = meta
    c2 = unalign_fb.shape[1]
    # per core: group-major [128, 7*260] -> per group (PC, 65)
    nd_all = np.zeros((7, N_CORES * PC, 65), np.float32)
    for i in range(N_CORES):
        o = np.asarray(res.results[i]["out_nd"]).astype(np.float32)
        arr = o.reshape(128, 7, 4, 65)
        for g in range(7):
            nd_all[g, i * PC:(i + 1) * PC] = np.concatenate(
                [arr[0:min(128, PC - b * 128), g, b, :] for b in range(4)])
    ga_p = ga[:, perm]
    combined = np.zeros((c2, N_CORES * PC), np.float32)
    for k in range(3):
        gs = [g for g in range(1, 8) if g & (1 << k)]
        numer = sum(nd_all[g - 1][:, 0:64] for g in gs)
        denom = sum(nd_all[g - 1][:, 64] for g in gs)
        numer = numer.T + vks[:, k:k + 1]
        denom = denom + zk[k]
        combined += (ga_p[k] / denom)[None, :] * numer
    aligned = np.zeros((c2, HW), np.float32)
    aligned[:, perm] = combined
    out = _interp_bilinear_ac(aligned.reshape(c2, 64, 64), (256, 256))
    return out[None], res


def kernel(**inputs):
    out, _ = _run(inputs)
    return out
